# revision 1
# baseline (speedup 1.0000x reference)
"""DeepSeekV2 MoE layer on 8 trn2 NeuronCores (expert-parallel).

Strategy (v5):
  - Host: gate softmax + group-limited top-k routing -> per-expert sorted token
    lists and combine weights (control data only; all heavy FLOPs on device).
  - Device (SPMD over 8 cores, 4 experts each):
      Routed phase: per expert, one transposed dma_gather of its CAP=896
      (sorted, padded) tokens; mm1/mm3 (fp16) -> silu*mul -> mm2 -> scale by
      combine weight; scatter-add split by token row into y_a (rows < RA) via
      a prefix scatter and y_b (rows >= RA) via a suffix scatter (the token
      lists are sorted, so rows < RA live in the list prefix).
      All transposing gathers precede the big collectives (Tile serializes
      xbar-transpose DMAs against in-flight collectives - an overlap killer).
      RS_a, RS_b: two back-to-back ReduceScatter(add) collectives, issued the
      moment the routed phase ends; both overlap the shared-expert phase.
      A small warmup ReduceScatter (after expert 0's gather) absorbs the
      ~200us first-collective cost.
      Shared experts (full SI) for this core's own 512 output rows; SH2 adds
      rs_a-dependent rows first, rs_b-dependent rows last so no add waits.
  - Host: reassemble row blocks -> [B, S, H].
"""
import sys

import numpy as np

sys.path.insert(0, "/opt/trn_rl_repo")

import concourse.bass as bass
import concourse.mybir as mybir
import concourse.tile as tile
from concourse import bacc
from concourse.bass_utils import run_bass_kernel_spmd

F32 = mybir.dt.float32
FP16 = mybir.dt.float16
I16 = mybir.dt.int16
AF = mybir.ActivationFunctionType
OP = mybir.AluOpType

N_GROUP, TOPK_GROUP, TOP_K = 8, 3, 6
NCORES = 8


def _routing(x, gate_w):
    T, E = x.shape[0], gate_w.shape[0]
    logits = (x @ gate_w.T).astype(np.float64)
    e = np.exp(logits - logits.max(-1, keepdims=True))
    scores = e / e.sum(-1, keepdims=True)
    per_group = E // N_GROUP
    group_scores = scores.reshape(T, N_GROUP, per_group).max(-1)
    order = np.argsort(-group_scores, axis=-1, kind="stable")
    group_mask = np.zeros((T, N_GROUP), bool)
    np.put_along_axis(group_mask, order[:, :TOPK_GROUP], True, axis=1)
    tmp = np.where(np.repeat(group_mask, per_group, axis=1), scores, 0.0)
    order_e = np.argsort(-tmp, axis=-1, kind="stable")
    topk_idx = order_e[:, :TOP_K]
    topk_w = np.take_along_axis(tmp, topk_idx, axis=1)
    topk_w = topk_w / (topk_w.sum(-1, keepdims=True) + 1e-20)
    combine = np.zeros((T, E), np.float32)
    np.put_along_axis(combine, topk_idx, topk_w.astype(np.float32), axis=1)
    return combine


def _wrap16(a):
    """[n] int16 -> [128, n//16] index layout for dma_gather/scatter."""
    return np.tile(a.reshape(-1, 16).T, (8, 1))


def _chunks(cap):
    out, rem = [], cap
    while rem:
        if rem <= 512:
            out.append(rem)
            rem = 0
        elif rem == 640:
            out.append(384)
            rem = 256
        else:
            out.append(512)
            rem -= 512
    return out


def build_kernel(T, H, I, EPC, CAP, PRE, SUF0, RA, SI, act=AF.Silu,
                 compile_=True):
    KT = H // 128          # contraction tiles over H
    MT = I // 128          # I tiles
    CT = CAP // 128
    RB = T - RA
    SIT = SI // 128
    TOUT = T // NCORES     # own output rows (RA/8 + RB/8)
    TS = TOUT // 128
    NSTR = H // 512
    CHUNKS = _chunks(CAP)
    SUFN = CAP - SUF0      # suffix scatter length

    nc = bacc.Bacc("TRN2")
    x16 = nc.dram_tensor("x16", [T, H], FP16, kind="ExternalInput")
    xTc = nc.dram_tensor("xTc", [128, KT * TOUT], FP16, kind="ExternalInput")
    w13 = nc.dram_tensor("w13", [EPC, MT, 128, KT * 256], FP16, kind="ExternalInput")
    w2b = nc.dram_tensor("w2b", [EPC, 4, 128, MT * 512], FP16, kind="ExternalInput")
    sw13 = nc.dram_tensor("sw13", [SIT, 128, KT * 256], FP16, kind="ExternalInput")
    sw2b = nc.dram_tensor("sw2b", [2 * NSTR, 128, SIT * 256], FP16, kind="ExternalInput")
    idxg = nc.dram_tensor("idxg", [EPC, 128, CAP // 16], I16, kind="ExternalInput")
    idxsa = nc.dram_tensor("idxsa", [EPC, 128, PRE // 16], I16, kind="ExternalInput")
    idxsb = nc.dram_tensor("idxsb", [EPC, 128, SUFN // 16], I16, kind="ExternalInput")
    gat = nc.dram_tensor("gat", [EPC, 128, CT], F32, kind="ExternalInput")
    zeros = nc.dram_tensor("zeros", [max(RA, RB), H], FP16, kind="ExternalInput")
    out = nc.dram_tensor("out", [TOUT, H], FP16, kind="ExternalOutput")

    y_a = nc.dram_tensor("y_a", [RA + 128, H], FP16)
    y_b = nc.dram_tensor("y_b", [RB + 128, H], FP16)
    rs_a = nc.dram_tensor("rs_a", [RA // NCORES, H], FP16)
    rs_b = nc.dram_tensor("rs_b", [RB // NCORES, H], FP16)
    warm_in = nc.dram_tensor("warm_in", [2048, 512], FP16)
    warm_out = nc.dram_tensor("warm_out", [256, 512], FP16)
    hb_in = nc.dram_tensor("hb_in", [MT * CAP // 512 * 128, 512], FP16)
    hb_out = nc.dram_tensor("hb_out", [MT * CAP // 512 * 16, 512], FP16)

    grp = [list(range(NCORES))]

    with tile.TileContext(nc) as tc:
        with (
            tc.tile_pool(name="const", bufs=1) as const,
            tc.tile_pool(name="persist", bufs=1) as persist,
            tc.tile_pool(name="xgtp", bufs=2) as xgtp,
            tc.tile_pool(name="xgtp1", bufs=2) as xgtp1,
            tc.tile_pool(name="gp", bufs=2) as gp,
            tc.tile_pool(name="w13p", bufs=2) as w13p,
            tc.tile_pool(name="w2p", bufs=2) as w2p,
            tc.tile_pool(name="ybp", bufs=2) as ybp,
            tc.tile_pool(name="s13p", bufs=2) as s13p,
            tc.tile_pool(name="s2p", bufs=2) as s2p,
            tc.tile_pool(name="small", bufs=2) as small,
            tc.tile_pool(name="psum", bufs=2, space="PSUM") as psum,
        ):
            # ---------------- constants ------------------------------------
            iga = const.tile([128, EPC, CAP // 16], I16)
            nc.sync.dma_start(iga[:], idxg.rearrange("e p c -> p e c"))
            isa = const.tile([128, EPC, PRE // 16], I16)
            nc.sync.dma_start(isa[:], idxsa.rearrange("e p c -> p e c"))
            isb = const.tile([128, EPC, SUFN // 16], I16)
            nc.sync.dma_start(isb[:], idxsb.rearrange("e p c -> p e c"))
            ga_sb = const.tile([128, EPC, CT], F32)
            nc.sync.dma_start(ga_sb[:], gat.rearrange("e p c -> p e c"))

            # zero y via few big dram->dram copies (kick-count on the queue
            # matters: 100+ small kicks starved the first silu for ~100us)
            QR = RA // 4
            for b in range(4):
                nc.scalar.dma_start(y_a[b * QR:(b + 1) * QR, :],
                                    zeros[b * QR:(b + 1) * QR, :])
            QB = RB // 4
            for b in range(4):
                nc.scalar.dma_start(y_b[b * QB:(b + 1) * QB, :],
                                    zeros[b * QB:(b + 1) * QB, :])
            nc.scalar.dma_start(warm_in[:], zeros[0:2048, 0:512])

            # shared-expert input tokens (used at the end)
            xtc_sb = persist.tile([128, KT, TOUT], FP16)
            nc.scalar.dma_start(xtc_sb[:], xTc.rearrange("p (k t) -> p k t", t=TOUT))
            gs = persist.tile([128, SIT, TOUT], FP16)

            # ---------------- routed experts -------------------------------
            for e in range(EPC):
                xgt_c = []
                c0 = 0
                for ci, cw in enumerate(CHUNKS):
                    pool_ci = xgtp if ci == 0 else xgtp1
                    xgt = pool_ci.tile([128, KT, cw], FP16, tag=f"xgt{ci}",
                                       name=f"xgt{ci}")
                    nc.gpsimd.dma_gather(
                        xgt[:], x16[:], iga[:, e, c0 // 16:(c0 + cw) // 16],
                        cw, cw, H, transpose=True)
                    xgt_c.append(xgt)
                    c0 += cw
                if e == 0:
                    # warmup collective: after the first gather so it doesn't
                    # delay it (transpose DMAs serialize with collectives)
                    nc.gpsimd.collective_compute(
                        "ReduceScatter", OP.add, replica_groups=grp,
                        ins=[warm_in[:]], outs=[warm_out[:]])
                g = gp.tile([128, MT, CAP], FP16, tag="g")
                for m in range(MT):
                    w13t = w13p.tile([128, KT, 256], FP16, tag="w13t")
                    nc.sync.dma_start(
                        w13t[:], w13[e, m].rearrange("p (k c) -> p k c", c=256))
                    c0 = 0
                    for ci, cw in enumerate(CHUNKS):
                        p1 = psum.tile([128, 512], F32, tag="p1")
                        p3 = psum.tile([128, 512], F32, tag="p3")
                        for k in range(KT):
                            nc.tensor.matmul(p1[:, :cw], w13t[:, k, :128],
                                             xgt_c[ci][:, k, :],
                                             start=(k == 0), stop=(k == KT - 1))
                        for k in range(KT):
                            nc.tensor.matmul(p3[:, :cw], w13t[:, k, 128:],
                                             xgt_c[ci][:, k, :],
                                             start=(k == 0), stop=(k == KT - 1))
                        nc.scalar.activation(g[:, m, c0:c0 + cw], p1[:, :cw], act)
                        nc.vector.tensor_tensor(g[:, m, c0:c0 + cw],
                                                g[:, m, c0:c0 + cw],
                                                p3[:, :cw], OP.mult)
                        c0 += cw
                if e == EPC - 1:
                    # single keep-warm heartbeat tied to the last expert's g
                    # (real data dep) so RS_a right after it runs warm
                    # (~80-97GB/s vs ~25GB/s after a long CC-idle gap)
                    nc.scalar.dma_start(
                        hb_in.rearrange("(c p) w -> p c w", p=128), g[:])
                    nc.gpsimd.collective_compute(
                        "ReduceScatter", OP.add, replica_groups=grp,
                        ins=[hb_in[:]], outs=[hb_out[:]])
                for q in range(4):
                    w2t = w2p.tile([128, MT, 512], FP16, tag="w2t")
                    nc.sync.dma_start(
                        w2t[:], w2b[e, q].rearrange("p (k c) -> p k c", c=512))
                    yb = ybp.tile([128, CT, 512], FP16, tag="yb")
                    for ct in range(CT):
                        p4a = psum.tile([128, 512], F32, tag="p4a")
                        for k2 in range(MT):
                            nc.tensor.matmul(p4a[:], g[:, k2, ct * 128:(ct + 1) * 128],
                                             w2t[:, k2, :],
                                             start=(k2 == 0), stop=(k2 == MT - 1))
                        nc.vector.tensor_tensor(
                            yb[:, ct, :], p4a[:],
                            ga_sb[:, e, ct:ct + 1].to_broadcast([128, 512]),
                            OP.mult)
                    # prefix -> y_a (rows < RA; overflow entries to pad row RA)
                    nc.gpsimd.dma_scatter_add(
                        y_a[:, q * 512:(q + 1) * 512], yb[:, :PRE // 128, :],
                        isa[:, e, :], PRE, PRE, 512, elem_step=H)
                    # suffix -> y_b (rows >= RA; others to pad row RB)
                    nc.gpsimd.dma_scatter_add(
                        y_b[:, q * 512:(q + 1) * 512], yb[:, SUF0 // 128:, :],
                        isb[:, e, :], SUFN, SUFN, 512, elem_step=H)

            nc.gpsimd.collective_compute(
                "ReduceScatter", OP.add, replica_groups=grp,
                ins=[y_a[0:RA, :]], outs=[rs_a[:]])
            nc.gpsimd.collective_compute(
                "ReduceScatter", OP.add, replica_groups=grp,
                ins=[y_b[0:RB, :]], outs=[rs_b[:]])

            # ---------------- shared experts (own rows) --------------------
            for sm in range(SIT):
                s13 = s13p.tile([128, KT, 256], FP16, tag="s13")
                nc.scalar.dma_start(
                    s13[:], sw13[sm].rearrange("p (k c) -> p k c", c=256))
                p1 = psum.tile([128, 512], F32, tag="p1")
                p3 = psum.tile([128, 512], F32, tag="p3")
                for k in range(KT):
                    nc.tensor.matmul(p1[:, :TOUT], s13[:, k, :128], xtc_sb[:, k, :],
                                     start=(k == 0), stop=(k == KT - 1))
                for k in range(KT):
                    nc.tensor.matmul(p3[:, :TOUT], s13[:, k, 128:], xtc_sb[:, k, :],
                                     start=(k == 0), stop=(k == KT - 1))
                nc.scalar.activation(gs[:, sm, :], p1[:, :TOUT], act)
                nc.vector.tensor_tensor(gs[:, sm, :], gs[:, sm, :], p3[:, :TOUT],
                                        OP.mult)

            # shared out per 256-col strip + combine with rs halves.
            # Process rs_a-dependent rows (ts < TS/2) across all strips first,
            # then rs_b rows, so no add waits on the later collective.
            half_ts = TS // 2
            for rgn in range(2):
                for s in range(2 * NSTR):
                    s2 = s2p.tile([128, SIT, 256], FP16, tag="s2")
                    nc.scalar.dma_start(
                        s2[:], sw2b[s].rearrange("p (k c) -> p k c", c=256))
                    for tsr in range(half_ts):
                        ts = rgn * half_ts + tsr
                        po = psum.tile([128, 256], F32, tag="p4a")
                        for k2 in range(SIT):
                            nc.tensor.matmul(po[:], gs[:, k2, ts * 128:(ts + 1) * 128],
                                             s2[:, k2, :],
                                             start=(k2 == 0), stop=(k2 == SIT - 1))
                        rst = small.tile([128, 256], FP16, tag="rst")
                        if rgn == 0:
                            src = rs_a[tsr * 128:(tsr + 1) * 128,
                                       s * 256:(s + 1) * 256]
                        else:
                            src = rs_b[tsr * 128:(tsr + 1) * 128,
                                       s * 256:(s + 1) * 256]
                        # on sync: a collective-gated DMA kick must not sit in
                        # front of s13/s2 loads on the in-order scalar queue
                        nc.sync.dma_start(rst[:], src)
                        ott = small.tile([128, 256], FP16, tag="ott")
                        nc.vector.tensor_tensor(ott[:], po[:], rst[:], OP.add)
                        nc.sync.dma_start(
                            out[ts * 128:(ts + 1) * 128, s * 256:(s + 1) * 256],
                            ott[:])

    if compile_:
        nc.compile()
    else:
        nc.insert_library_loads()
    return nc


def host_prep(hidden_states, gate_weight, w1, w2, w3, sw1, sw2, sw3):
    B, S, H = hidden_states.shape
    T = B * S
    E, I = w1.shape[0], w1.shape[1]
    SI = sw1.shape[0]
    EPC = E // NCORES
    KT, MT, SIT = H // 128, I // 128, SI // 128
    NSTR = H // 512

    x = np.ascontiguousarray(hidden_states.reshape(T, H), dtype=np.float32)
    combine = _routing(x, gate_weight.astype(np.float32))
    tok_lists = [np.nonzero(combine[:, e])[0] for e in range(E)]
    counts = np.array([len(t) for t in tok_lists])

    CAP = max(256, ((counts.max() + 127) // 128) * 128)
    RA = T // 2
    counts_a = np.array([(t < RA).sum() for t in tok_lists])
    # prefix must contain every token < RA; suffix every token >= RA
    PRE = min(CAP, ((int(counts_a.max()) + 127) // 128) * 128)
    SUF0 = int(counts_a.min()) // 128 * 128
    RB = T - RA

    x16 = x.astype(np.float16)
    xT = x.T  # [H, T] view

    s1 = sw1.T.reshape(KT, 128, SIT, 128).transpose(2, 1, 0, 3)
    s3 = sw3.T.reshape(KT, 128, SIT, 128).transpose(2, 1, 0, 3)
    sw13 = np.ascontiguousarray(
        np.concatenate([s1, s3], axis=-1).reshape(SIT, 128, -1), dtype=np.float16)
    sw2b = np.ascontiguousarray(
        sw2.T.reshape(SIT, 128, 2 * NSTR, 256).transpose(2, 1, 0, 3)
        .reshape(2 * NSTR, 128, -1), dtype=np.float16)

    zeros_buf = np.zeros((max(RA, RB), H), np.float16)
    in_maps = []
    for c in range(NCORES):
        els = list(range(c * EPC, (c + 1) * EPC))
        idxg = np.zeros((EPC, 128, CAP // 16), np.int16)
        idxsa = np.zeros((EPC, 128, PRE // 16), np.int16)
        idxsb = np.zeros((EPC, 128, (CAP - SUF0) // 16), np.int16)
        gat = np.zeros((EPC, 128, CAP // 128), np.float32)
        for j, e in enumerate(els):
            toks = tok_lists[e]
            n = len(toks)
            na = int(counts_a[e])
            full = np.zeros(CAP, np.int16)
            full[:n] = toks
            idxg[j] = _wrap16(full)
            # prefix scatter: rows < RA keep their index, others -> pad row RA
            sa = np.full(PRE, RA, np.int16)
            sa[:na] = toks[:na]
            idxsa[j] = _wrap16(sa)
            # suffix scatter: rows >= RA -> local index, others -> pad row RB
            sb = np.full(CAP - SUF0, RB, np.int16)
            sel = full[SUF0:].astype(np.int64)
            vmask = (np.arange(SUF0, CAP) < n) & (sel >= RA)
            sb[vmask] = (sel[vmask] - RA).astype(np.int16)
            idxsb[j] = _wrap16(sb)
            gv = np.zeros(CAP, np.float32)
            gv[:n] = combine[toks, e]
            gat[j] = gv.reshape(-1, 128).T
        w13c = np.empty((EPC, MT, 128, KT * 256), np.float16)
        w2c = np.empty((EPC, 4, 128, MT * 512), np.float16)
        for j, e in enumerate(els):
            a1 = w1[e].T.reshape(KT, 128, MT, 128).transpose(2, 1, 0, 3)
            a3 = w3[e].T.reshape(KT, 128, MT, 128).transpose(2, 1, 0, 3)
            w13c[j] = np.concatenate([a1, a3], axis=-1).reshape(MT, 128, -1)
            w2c[j] = (w2[e].T.reshape(MT, 128, 4, 512)
                      .transpose(2, 1, 0, 3).reshape(4, 128, -1))
        own_rows = np.concatenate([
            np.arange(c * RA // NCORES, (c + 1) * RA // NCORES),
            np.arange(RA + c * RB // NCORES, RA + (c + 1) * RB // NCORES)])
        xTc = np.ascontiguousarray(
            xT[:, own_rows].reshape(KT, 128, len(own_rows))
            .transpose(1, 0, 2).reshape(128, -1), dtype=np.float16)
        in_maps.append({
            "x16": x16, "xTc": xTc,
            "w13": w13c, "w2b": w2c,
            "sw13": sw13, "sw2b": sw2b,
            "idxg": idxg, "idxsa": idxsa, "idxsb": idxsb,
            "gat": gat, "zeros": zeros_buf,
        })
    cfg = dict(T=T, H=H, I=I, EPC=EPC, CAP=CAP, PRE=PRE, SUF0=SUF0, RA=RA,
               SI=SI)
    return in_maps, cfg


def kernel(**inputs):
    inputs = {k: np.asarray(v) for k, v in inputs.items()}
    hs = inputs["hidden_states"]
    B, S, H = hs.shape
    in_maps, cfg = host_prep(
        hs, inputs["gate_weight"], inputs["w1"], inputs["w2"], inputs["w3"],
        inputs["sw1"], inputs["sw2"], inputs["sw3"])
    nc = build_kernel(**cfg)
    res = run_bass_kernel_spmd(nc, in_maps, list(range(NCORES)))
    T = B * S
    RA = cfg["RA"]
    RB = T - RA
    y = np.empty((T, H), np.float32)
    for c in range(NCORES):
        o = res.results[c]["out"]
        y[c * RA // NCORES:(c + 1) * RA // NCORES] = o[:RA // NCORES]
        y[RA + c * RB // NCORES:RA + (c + 1) * RB // NCORES] = o[RA // NCORES:]
    return y.reshape(B, S, H).astype(np.float32)


if __name__ == "__main__":
    pass



# revision 6
# speedup vs baseline: 1.1311x; 1.1311x over previous
"""DeepSeekV2 MoE layer on 8 trn2 NeuronCores (expert-parallel).

Strategy (v6):
  - Host: gate softmax + group-limited top-k routing -> per-expert sorted token
    lists and combine weights (control data only; all heavy FLOPs on device).
  - Experts are rank-matched to (core, slot): sort by token count desc, slot j
    holds ranks [8j, 8j+8) so slot capacity = count of its largest expert.
    This trims ~7-10% of the padded matmul rows vs one global CAP.
  - Device (SPMD over 8 cores, 4 expert slots each):
      Routed phase: per slot, transposed dma_gathers of its CAP_j tokens in
      chunks [256, 512, ...] (small first chunk -> first matmul starts early),
      on the single Tile-managed swdge queue (multi-queue swdge is
      racy: sem assignment is queue-unaware); mm1/mm3 (fp16) -> silu*mul -> mm2
      -> scale by combine weight; scatter-add split by token row into y_a
      (rows < RA) and y_b (rows >= RA).
      Weight loads (w13/w2/shared) are split in half across the two hwdge
      queues (sync + scalar) - a single queue can't sustain the stream and
      stalls the PE (the v5 84us startup stall).
      y_a/y_b zeroing is 32 SBUF->DRAM writes from a memset tile emitted
      after expert 0's mm13 issue (write-only, no HBM reads; off the
      critical early weight-load window).
      RS_a, RS_b: two back-to-back ReduceScatter(add) collectives with
      addr_space="Shared" outputs; both overlap the shared-expert phase.
      A small warmup ReduceScatter (after expert 0's first gather) absorbs
      the ~200us first-collective cost; a heartbeat RS tied to the last
      expert's g keeps the fabric warm right before RS_a.
      Shared experts (full SI) for this core's own 512 output rows; SH2 adds
      rs_a-dependent rows first, rs_b-dependent rows last so no add waits.
  - Host: reassemble row blocks -> [B, S, H].
"""
import sys

import numpy as np

sys.path.insert(0, "/opt/trn_rl_repo")

import concourse.bass as bass
import concourse.mybir as mybir
import concourse.tile as tile
from concourse import bacc
from concourse.bass_utils import run_bass_kernel_spmd

F32 = mybir.dt.float32
FP16 = mybir.dt.float16
I16 = mybir.dt.int16
AF = mybir.ActivationFunctionType
OP = mybir.AluOpType

N_GROUP, TOPK_GROUP, TOP_K = 8, 3, 6
NCORES = 8


def _routing(x, gate_w):
    T, E = x.shape[0], gate_w.shape[0]
    logits = (x @ gate_w.T).astype(np.float64)
    e = np.exp(logits - logits.max(-1, keepdims=True))
    scores = e / e.sum(-1, keepdims=True)
    per_group = E // N_GROUP
    group_scores = scores.reshape(T, N_GROUP, per_group).max(-1)
    order = np.argsort(-group_scores, axis=-1, kind="stable")
    group_mask = np.zeros((T, N_GROUP), bool)
    np.put_along_axis(group_mask, order[:, :TOPK_GROUP], True, axis=1)
    tmp = np.where(np.repeat(group_mask, per_group, axis=1), scores, 0.0)
    order_e = np.argsort(-tmp, axis=-1, kind="stable")
    topk_idx = order_e[:, :TOP_K]
    topk_w = np.take_along_axis(tmp, topk_idx, axis=1)
    topk_w = topk_w / (topk_w.sum(-1, keepdims=True) + 1e-20)
    combine = np.zeros((T, E), np.float32)
    np.put_along_axis(combine, topk_idx, topk_w.astype(np.float32), axis=1)
    return combine


def _wrap16(a):
    """[n] int16 -> [128, n//16] index layout for dma_gather/scatter."""
    return np.tile(a.reshape(-1, 16).T, (8, 1))


def _chunks(cap):
    """[256, 512..., 128-tail...] - uniform structure so chunk-index tags
    have one shape across slots; small first chunk for fast pipeline fill."""
    out = [min(256, cap)]
    rem = cap - out[0]
    while rem >= 512:
        out.append(512)
        rem -= 512
    while rem:
        out.append(128)
        rem -= 128
    return out


def build_kernel(T, H, I, EPC, CAPS, PRES, SUF0S, RA, SI, act=AF.Silu,
                 compile_=True):
    KT = H // 128          # contraction tiles over H
    MT = I // 128          # I tiles
    RB = T - RA
    SIT = SI // 128
    TOUT = T // NCORES     # own output rows (RA/8 + RB/8)
    TS = TOUT // 128
    NSTR = H // 512
    CAPM = max(CAPS)
    CTM = CAPM // 128
    PREM = max(PRES)
    SUFNS = [CAPS[j] - SUF0S[j] for j in range(EPC)]
    SUFM = max(SUFNS)
    CHUNKS = [_chunks(c) for c in CAPS]

    nc = bacc.Bacc("TRN2")
    x16 = nc.dram_tensor("x16", [T, H], FP16, kind="ExternalInput")
    xTc = nc.dram_tensor("xTc", [128, KT * TOUT], FP16, kind="ExternalInput")
    w13 = nc.dram_tensor("w13", [EPC, MT, 128, KT * 256], FP16, kind="ExternalInput")
    w2b = nc.dram_tensor("w2b", [EPC, 4, 128, MT * 512], FP16, kind="ExternalInput")
    sw13 = nc.dram_tensor("sw13", [SIT, 128, KT * 256], FP16, kind="ExternalInput")
    sw2b = nc.dram_tensor("sw2b", [2 * NSTR, 128, SIT * 256], FP16, kind="ExternalInput")
    idxg = nc.dram_tensor("idxg", [EPC, 128, CAPM // 16], I16, kind="ExternalInput")
    idxsa = nc.dram_tensor("idxsa", [EPC, 128, PREM // 16], I16, kind="ExternalInput")
    idxsb = nc.dram_tensor("idxsb", [EPC, 128, SUFM // 16], I16, kind="ExternalInput")
    gat = nc.dram_tensor("gat", [EPC, 128, CTM], F32, kind="ExternalInput")
    out = nc.dram_tensor("out", [TOUT, H], FP16, kind="ExternalOutput")

    y_a = nc.dram_tensor("y_a", [RA + 128, H], FP16)
    y_b = nc.dram_tensor("y_b", [RB + 128, H], FP16)
    rs_a = nc.dram_tensor("rs_a", [RA // NCORES, H], FP16)
    rs_b = nc.dram_tensor("rs_b", [RB // NCORES, H], FP16)
    warm_in = nc.dram_tensor("warm_in", [2048, 512], FP16)
    warm_out = nc.dram_tensor("warm_out", [256, 512], FP16)
    hb_in = nc.dram_tensor("hb_in", [MT * CAPS[-1] // 512 * 128, 512], FP16)
    hb_out = nc.dram_tensor("hb_out", [MT * CAPS[-1] // 512 * 16, 512], FP16)

    grp = [list(range(NCORES))]

    with tile.TileContext(nc) as tc:
        with (
            tc.tile_pool(name="const", bufs=1) as const,
            tc.tile_pool(name="persist", bufs=1) as persist,
            tc.tile_pool(name="xgtp", bufs=2) as xgtp,
            tc.tile_pool(name="xgtp1", bufs=2) as xgtp1,
            tc.tile_pool(name="gp", bufs=2) as gp,
            tc.tile_pool(name="w13p", bufs=2) as w13p,
            tc.tile_pool(name="w2p", bufs=2) as w2p,
            tc.tile_pool(name="ybp", bufs=2) as ybp,
            tc.tile_pool(name="s13p", bufs=2) as s13p,
            tc.tile_pool(name="s2p", bufs=2) as s2p,
            tc.tile_pool(name="small", bufs=2) as small,
            tc.tile_pool(name="psum", bufs=2, space="PSUM") as psum,
        ):
            # ---------------- constants ------------------------------------
            iga = const.tile([128, EPC, CAPM // 16], I16)
            nc.sync.dma_start(iga[:], idxg.rearrange("e p c -> p e c"))
            isa = const.tile([128, EPC, PREM // 16], I16)
            nc.sync.dma_start(isa[:], idxsa.rearrange("e p c -> p e c"))
            isb = const.tile([128, EPC, SUFM // 16], I16)
            nc.sync.dma_start(isb[:], idxsb.rearrange("e p c -> p e c"))
            ga_sb = const.tile([128, EPC, CTM], F32)
            nc.sync.dma_start(ga_sb[:], gat.rearrange("e p c -> p e c"))
            zt = const.tile([128, 2048], FP16)
            nc.vector.memset(zt[:], 0.0)

            gs = persist.tile([128, SIT, TOUT], FP16)
            xtc_sb = persist.tile([128, KT, TOUT], FP16)

            # ---------------- routed experts -------------------------------
            for e in range(EPC):
                CH = CHUNKS[e]
                CAP, PRE, SUF0 = CAPS[e], PRES[e], SUF0S[e]
                SUFN = SUFNS[e]
                CT = CAP // 128
                xgt_c = []
                c0 = 0
                for ci, cw in enumerate(CH):
                    pool_ci = xgtp if ci == 0 else xgtp1
                    xgt = pool_ci.tile([128, KT, cw], FP16, tag=f"xgt{ci}",
                                       name=f"xgt{ci}")
                    nc.gpsimd.dma_gather(
                        xgt[:], x16[:], iga[:, e, c0 // 16:(c0 + cw) // 16],
                        cw, cw, H, transpose=True)
                    xgt_c.append(xgt)
                    c0 += cw
                if e == 0:
                    # warmup collective: after the first gather so it doesn't
                    # delay it (transpose DMAs serialize with collectives).
                    # warm_in is uninitialized garbage - result is discarded.
                    nc.gpsimd.collective_compute(
                        "ReduceScatter", OP.add, replica_groups=grp,
                        ins=[warm_in[:]], outs=[warm_out[:]])
                g = gp.tile([128, MT, CAPM], FP16, tag="g")
                for m in range(MT):
                    w13ta = w13p.tile([128, KT // 2, 256], FP16, tag="w13ta")
                    w13tb = w13p.tile([128, KT // 2, 256], FP16, tag="w13tb")
                    w13s = w13[e, m].rearrange("p (k c) -> p k c", c=256)
                    nc.sync.dma_start(w13ta[:], w13s[:, :KT // 2, :])
                    nc.scalar.dma_start(w13tb[:], w13s[:, KT // 2:, :])
                    c0 = 0
                    for ci, cw in enumerate(CH):
                        p1 = psum.tile([128, 512], F32, tag="p1")
                        p3 = psum.tile([128, 512], F32, tag="p3")
                        for k in range(KT):
                            wh = w13ta if k < KT // 2 else w13tb
                            nc.tensor.matmul(p1[:, :cw], wh[:, k % (KT // 2), :128],
                                             xgt_c[ci][:, k, :],
                                             start=(k == 0), stop=(k == KT - 1))
                        for k in range(KT):
                            wh = w13ta if k < KT // 2 else w13tb
                            nc.tensor.matmul(p3[:, :cw], wh[:, k % (KT // 2), 128:],
                                             xgt_c[ci][:, k, :],
                                             start=(k == 0), stop=(k == KT - 1))
                        nc.scalar.activation(g[:, m, c0:c0 + cw], p1[:, :cw], act)
                        nc.vector.tensor_tensor(g[:, m, c0:c0 + cw],
                                                g[:, m, c0:c0 + cw],
                                                p3[:, :cw], OP.mult)
                        c0 += cw
                if e == 0:
                    # zero y_a/y_b from the SBUF memset tile: write-only HBM
                    # traffic, emitted after e0's w13 kicks so the first
                    # weight loads aren't stuck behind 16.8MB of zeros.
                    for r in range(0, RA, 256):
                        nc.sync.dma_start(y_a[r:r + 128, :], zt[:])
                        nc.scalar.dma_start(y_a[r + 128:r + 256, :], zt[:])
                    for r in range(0, RB, 256):
                        nc.sync.dma_start(y_b[r:r + 128, :], zt[:])
                        nc.scalar.dma_start(y_b[r + 128:r + 256, :], zt[:])
                if e == 2:
                    # shared-expert input tokens (needed only at the end;
                    # emitted here to keep it off the startup-critical queues)
                    nc.scalar.dma_start(
                        xtc_sb[:], xTc.rearrange("p (k t) -> p k t", t=TOUT))
                if e == EPC - 1:
                    # single keep-warm heartbeat tied to the last expert's g
                    # (real data dep) so RS_a right after it runs warm
                    hbn = hb_in.shape[0] // 128 * 512
                    nc.scalar.dma_start(
                        hb_in.rearrange("(c p) w -> p c w", p=128),
                        g[:].rearrange("p k c -> p (k c)")[:, :hbn]
                        .rearrange("p (a b) -> p a b", b=512))
                    nc.gpsimd.collective_compute(
                        "ReduceScatter", OP.add, replica_groups=grp,
                        ins=[hb_in[:]], outs=[hb_out[:]])
                for q in range(4):
                    w2ta = w2p.tile([128, MT // 2, 512], FP16, tag="w2ta")
                    w2tb = w2p.tile([128, MT // 2, 512], FP16, tag="w2tb")
                    w2s = w2b[e, q].rearrange("p (k c) -> p k c", c=512)
                    nc.sync.dma_start(w2ta[:], w2s[:, :MT // 2, :])
                    nc.scalar.dma_start(w2tb[:], w2s[:, MT // 2:, :])
                    yb = ybp.tile([128, CTM, 512], FP16, tag="yb")
                    for ct in range(CT):
                        p4a = psum.tile([128, 512], F32, tag="p4a")
                        for k2 in range(MT):
                            w2h = w2ta if k2 < MT // 2 else w2tb
                            nc.tensor.matmul(p4a[:], g[:, k2, ct * 128:(ct + 1) * 128],
                                             w2h[:, k2 % (MT // 2), :],
                                             start=(k2 == 0), stop=(k2 == MT - 1))
                        nc.vector.tensor_tensor(
                            yb[:, ct, :], p4a[:],
                            ga_sb[:, e, ct:ct + 1].to_broadcast([128, 512]),
                            OP.mult)
                    # prefix -> y_a (rows < RA; overflow entries to pad row RA)
                    nc.gpsimd.dma_scatter_add(
                        y_a[:, q * 512:(q + 1) * 512], yb[:, :PRE // 128, :],
                        isa[:, e, :PRE // 16], PRE, PRE, 512, elem_step=H)
                    # suffix -> y_b (rows >= RA; others to pad row RB)
                    nc.gpsimd.dma_scatter_add(
                        y_b[:, q * 512:(q + 1) * 512], yb[:, SUF0 // 128:CT, :],
                        isb[:, e, :SUFN // 16], SUFN, SUFN, 512, elem_step=H)

            nc.gpsimd.collective_compute(
                "ReduceScatter", OP.add, replica_groups=grp,
                ins=[y_a[0:RA, :]], outs=[rs_a[:]])
            nc.gpsimd.collective_compute(
                "ReduceScatter", OP.add, replica_groups=grp,
                ins=[y_b[0:RB, :]], outs=[rs_b[:]])

            # ---------------- shared experts (own rows) --------------------
            for sm in range(SIT):
                s13a = s13p.tile([128, KT // 2, 256], FP16, tag="s13a")
                s13b = s13p.tile([128, KT // 2, 256], FP16, tag="s13b")
                s13s = sw13[sm].rearrange("p (k c) -> p k c", c=256)
                nc.sync.dma_start(s13a[:], s13s[:, :KT // 2, :])
                nc.scalar.dma_start(s13b[:], s13s[:, KT // 2:, :])
                p1 = psum.tile([128, 512], F32, tag="p1")
                p3 = psum.tile([128, 512], F32, tag="p3")
                for k in range(KT):
                    sh = s13a if k < KT // 2 else s13b
                    nc.tensor.matmul(p1[:, :TOUT], sh[:, k % (KT // 2), :128],
                                     xtc_sb[:, k, :],
                                     start=(k == 0), stop=(k == KT - 1))
                for k in range(KT):
                    sh = s13a if k < KT // 2 else s13b
                    nc.tensor.matmul(p3[:, :TOUT], sh[:, k % (KT // 2), 128:],
                                     xtc_sb[:, k, :],
                                     start=(k == 0), stop=(k == KT - 1))
                nc.scalar.activation(gs[:, sm, :], p1[:, :TOUT], act)
                nc.vector.tensor_tensor(gs[:, sm, :], gs[:, sm, :], p3[:, :TOUT],
                                        OP.mult)

            # shared out per 256-col strip + combine with rs halves.
            # Process rs_a-dependent rows (ts < TS/2) across all strips first,
            # then rs_b rows, so no add waits on the later collective.
            half_ts = TS // 2
            for rgn in range(2):
                for s in range(2 * NSTR):
                    s2a = s2p.tile([128, SIT // 2, 256], FP16, tag="s2a")
                    s2b = s2p.tile([128, SIT // 2, 256], FP16, tag="s2b")
                    s2s = sw2b[s].rearrange("p (k c) -> p k c", c=256)
                    nc.sync.dma_start(s2a[:], s2s[:, :SIT // 2, :])
                    nc.scalar.dma_start(s2b[:], s2s[:, SIT // 2:, :])
                    for tsr in range(half_ts):
                        ts = rgn * half_ts + tsr
                        po = psum.tile([128, 256], F32, tag="p4a")
                        for k2 in range(SIT):
                            s2h = s2a if k2 < SIT // 2 else s2b
                            nc.tensor.matmul(po[:], gs[:, k2, ts * 128:(ts + 1) * 128],
                                             s2h[:, k2 % (SIT // 2), :],
                                             start=(k2 == 0), stop=(k2 == SIT - 1))
                        rst = small.tile([128, 256], FP16, tag="rst")
                        if rgn == 0:
                            src = rs_a[tsr * 128:(tsr + 1) * 128,
                                       s * 256:(s + 1) * 256]
                        else:
                            src = rs_b[tsr * 128:(tsr + 1) * 128,
                                       s * 256:(s + 1) * 256]
                        # on sync: a collective-gated DMA kick must not sit in
                        # front of s13/s2 loads on the in-order scalar queue
                        nc.sync.dma_start(rst[:], src)
                        ott = small.tile([128, 256], FP16, tag="ott")
                        nc.vector.tensor_tensor(ott[:], po[:], rst[:], OP.add)
                        nc.sync.dma_start(
                            out[ts * 128:(ts + 1) * 128, s * 256:(s + 1) * 256],
                            ott[:])

    if compile_:
        nc.compile()
    else:
        nc.insert_library_loads()
    return nc


def host_prep(hidden_states, gate_weight, w1, w2, w3, sw1, sw2, sw3):
    B, S, H = hidden_states.shape
    T = B * S
    E, I = w1.shape[0], w1.shape[1]
    SI = sw1.shape[0]
    EPC = E // NCORES
    KT, MT, SIT = H // 128, I // 128, SI // 128
    NSTR = H // 512

    x = np.ascontiguousarray(hidden_states.reshape(T, H), dtype=np.float32)
    combine = _routing(x, gate_weight.astype(np.float32))
    tok_lists = [np.nonzero(combine[:, e])[0] for e in range(E)]
    counts = np.array([len(t) for t in tok_lists])

    # rank-matched expert assignment: sort by count desc; slot j holds ranks
    # [8j, 8j+8); core c gets order[8j + c]. Slot capacity covers its max.
    order = np.argsort(-counts, kind="stable")
    CAPS = [max(256, int(np.ceil(counts[order[8 * j]] / 128) * 128))
            for j in range(EPC)]

    RA = T // 2
    RB = T - RA
    counts_a = np.array([(t < RA).sum() for t in tok_lists])
    PRES, SUF0S = [], []
    for j in range(EPC):
        slot_experts = order[8 * j:8 * j + 8]
        ca = counts_a[slot_experts]
        PRES.append(min(CAPS[j], int(np.ceil(ca.max() / 128) * 128)))
        SUF0S.append(int(ca.min()) // 128 * 128)
    CAPM = max(CAPS)
    CTM = CAPM // 128
    PREM = max(PRES)
    SUFM = max(CAPS[j] - SUF0S[j] for j in range(EPC))

    x16 = x.astype(np.float16)
    xT = x.T  # [H, T] view

    s1 = sw1.T.reshape(KT, 128, SIT, 128).transpose(2, 1, 0, 3)
    s3 = sw3.T.reshape(KT, 128, SIT, 128).transpose(2, 1, 0, 3)
    sw13 = np.ascontiguousarray(
        np.concatenate([s1, s3], axis=-1).reshape(SIT, 128, -1), dtype=np.float16)
    sw2b = np.ascontiguousarray(
        sw2.T.reshape(SIT, 128, 2 * NSTR, 256).transpose(2, 1, 0, 3)
        .reshape(2 * NSTR, 128, -1), dtype=np.float16)

    in_maps = []
    for c in range(NCORES):
        els = [int(order[8 * j + c]) for j in range(EPC)]
        idxg = np.zeros((EPC, 128, CAPM // 16), np.int16)
        idxsa = np.zeros((EPC, 128, PREM // 16), np.int16)
        idxsb = np.zeros((EPC, 128, SUFM // 16), np.int16)
        gatv = np.zeros((EPC, 128, CTM), np.float32)
        for j, e in enumerate(els):
            CAP, PRE, SUF0 = CAPS[j], PRES[j], SUF0S[j]
            SUFN = CAP - SUF0
            toks = tok_lists[e]
            n = len(toks)
            na = int(counts_a[e])
            full = np.zeros(CAP, np.int16)
            full[:n] = toks
            idxg[j, :, :CAP // 16] = _wrap16(full)
            # prefix scatter: rows < RA keep their index, others -> pad row RA
            sa = np.full(PRE, RA, np.int16)
            sa[:na] = toks[:na]
            idxsa[j, :, :PRE // 16] = _wrap16(sa)
            # suffix scatter: rows >= RA -> local index, others -> pad row RB
            sb = np.full(SUFN, RB, np.int16)
            sel = full[SUF0:].astype(np.int64)
            vmask = (np.arange(SUF0, CAP) < n) & (sel >= RA)
            sb[vmask] = (sel[vmask] - RA).astype(np.int16)
            idxsb[j, :, :SUFN // 16] = _wrap16(sb)
            gv = np.zeros(CAP, np.float32)
            gv[:n] = combine[toks, e]
            gatv[j, :, :CAP // 128] = gv.reshape(-1, 128).T
        w13c = np.empty((EPC, MT, 128, KT * 256), np.float16)
        w2c = np.empty((EPC, 4, 128, MT * 512), np.float16)
        for j, e in enumerate(els):
            a1 = w1[e].T.reshape(KT, 128, MT, 128).transpose(2, 1, 0, 3)
            a3 = w3[e].T.reshape(KT, 128, MT, 128).transpose(2, 1, 0, 3)
            w13c[j] = np.concatenate([a1, a3], axis=-1).reshape(MT, 128, -1)
            w2c[j] = (w2[e].T.reshape(MT, 128, 4, 512)
                      .transpose(2, 1, 0, 3).reshape(4, 128, -1))
        own_rows = np.concatenate([
            np.arange(c * RA // NCORES, (c + 1) * RA // NCORES),
            np.arange(RA + c * RB // NCORES, RA + (c + 1) * RB // NCORES)])
        xTc = np.ascontiguousarray(
            xT[:, own_rows].reshape(KT, 128, len(own_rows))
            .transpose(1, 0, 2).reshape(128, -1), dtype=np.float16)
        in_maps.append({
            "x16": x16, "xTc": xTc,
            "w13": w13c, "w2b": w2c,
            "sw13": sw13, "sw2b": sw2b,
            "idxg": idxg, "idxsa": idxsa, "idxsb": idxsb,
            "gat": gatv,
        })
    cfg = dict(T=T, H=H, I=I, EPC=EPC, CAPS=CAPS, PRES=PRES, SUF0S=SUF0S,
               RA=RA, SI=SI)
    return in_maps, cfg


def kernel(**inputs):
    inputs = {k: np.asarray(v) for k, v in inputs.items()}
    hs = inputs["hidden_states"]
    B, S, H = hs.shape
    in_maps, cfg = host_prep(
        hs, inputs["gate_weight"], inputs["w1"], inputs["w2"], inputs["w3"],
        inputs["sw1"], inputs["sw2"], inputs["sw3"])
    nc = build_kernel(**cfg)
    res = run_bass_kernel_spmd(nc, in_maps, list(range(NCORES)))
    T = B * S
    RA = cfg["RA"]
    RB = T - RA
    y = np.empty((T, H), np.float32)
    for c in range(NCORES):
        o = res.results[c]["out"]
        y[c * RA // NCORES:(c + 1) * RA // NCORES] = o[:RA // NCORES]
        y[RA + c * RB // NCORES:RA + (c + 1) * RB // NCORES] = o[RA // NCORES:]
    return y.reshape(B, S, H).astype(np.float32)


if __name__ == "__main__":
    pass


# revision 7
# speedup vs baseline: 1.1469x; 1.0140x over previous
"""DeepSeekV2 MoE layer on 8 trn2 NeuronCores (expert-parallel).

Strategy (v6):
  - Host: gate softmax + group-limited top-k routing -> per-expert sorted token
    lists and combine weights (control data only; all heavy FLOPs on device).
  - Experts are rank-matched to (core, slot): sort by token count desc, slot j
    holds ranks [8j, 8j+8) so slot capacity = count of its largest expert.
    This trims ~7-10% of the padded matmul rows vs one global CAP.
  - Device (SPMD over 8 cores, 4 expert slots each):
      Routed phase: per slot, transposed dma_gathers of its CAP_j tokens in
      chunks [256, 512, ...] (small first chunk -> first matmul starts early),
      on the single Tile-managed swdge queue (multi-queue swdge is
      racy: sem assignment is queue-unaware); mm1/mm3 (fp16) -> silu*mul -> mm2
      -> scale by combine weight; scatter-add split by token row into y_a
      (rows < RA) and y_b (rows >= RA).
      Weight loads (w13/w2/shared) are split in half across the two hwdge
      queues (sync + scalar) - a single queue can't sustain the stream and
      stalls the PE (the v5 84us startup stall).
      y_a/y_b zeroing is 32 SBUF->DRAM writes from a memset tile emitted
      after expert 0's mm13 issue (write-only, no HBM reads; off the
      critical early weight-load window).
      RS_a, RS_b: two back-to-back ReduceScatter(add) collectives with
      addr_space="Shared" outputs; both overlap the shared-expert phase.
      A small warmup ReduceScatter (after expert 0's first gather) absorbs
      the ~200us first-collective cost; a heartbeat RS tied to the last
      expert's g keeps the fabric warm right before RS_a.
      Shared experts (full SI) for this core's own 512 output rows; SH2 adds
      rs_a-dependent rows first, rs_b-dependent rows last so no add waits.
  - Host: reassemble row blocks -> [B, S, H].
"""
import sys

import numpy as np

sys.path.insert(0, "/opt/trn_rl_repo")

import concourse.bass as bass
import concourse.mybir as mybir
import concourse.tile as tile
from concourse import bacc
from concourse.bass_utils import run_bass_kernel_spmd

F32 = mybir.dt.float32
FP16 = mybir.dt.float16
I16 = mybir.dt.int16
AF = mybir.ActivationFunctionType
OP = mybir.AluOpType

N_GROUP, TOPK_GROUP, TOP_K = 8, 3, 6
NCORES = 8


def _routing(x, gate_w):
    T, E = x.shape[0], gate_w.shape[0]
    logits = (x @ gate_w.T).astype(np.float64)
    e = np.exp(logits - logits.max(-1, keepdims=True))
    scores = e / e.sum(-1, keepdims=True)
    per_group = E // N_GROUP
    group_scores = scores.reshape(T, N_GROUP, per_group).max(-1)
    order = np.argsort(-group_scores, axis=-1, kind="stable")
    group_mask = np.zeros((T, N_GROUP), bool)
    np.put_along_axis(group_mask, order[:, :TOPK_GROUP], True, axis=1)
    tmp = np.where(np.repeat(group_mask, per_group, axis=1), scores, 0.0)
    order_e = np.argsort(-tmp, axis=-1, kind="stable")
    topk_idx = order_e[:, :TOP_K]
    topk_w = np.take_along_axis(tmp, topk_idx, axis=1)
    topk_w = topk_w / (topk_w.sum(-1, keepdims=True) + 1e-20)
    combine = np.zeros((T, E), np.float32)
    np.put_along_axis(combine, topk_idx, topk_w.astype(np.float32), axis=1)
    return combine


def _wrap16(a):
    """[n] int16 -> [128, n//16] index layout for dma_gather/scatter."""
    return np.tile(a.reshape(-1, 16).T, (8, 1))


def _chunks(cap):
    """[256, 512..., 128-tail...] - uniform structure so chunk-index tags
    have one shape across slots; small first chunk for fast pipeline fill."""
    out = [min(256, cap)]
    rem = cap - out[0]
    while rem >= 512:
        out.append(512)
        rem -= 512
    while rem:
        out.append(128)
        rem -= 128
    return out


def build_kernel(T, H, I, EPC, CAPS, PRES, SUF0S, RA, SI, act=AF.Silu,
                 compile_=True):
    KT = H // 128          # contraction tiles over H
    MT = I // 128          # I tiles
    RB = T - RA
    SIT = SI // 128
    TOUT = T // NCORES     # own output rows (RA/8 + RB/8)
    TS = TOUT // 128
    NSTR = H // 512
    CAPM = max(CAPS)
    CTM = CAPM // 128
    PREM = max(PRES)
    SUFNS = [CAPS[j] - SUF0S[j] for j in range(EPC)]
    SUFM = max(SUFNS)
    CHUNKS = [_chunks(c) for c in CAPS]

    nc = bacc.Bacc("TRN2")
    x16 = nc.dram_tensor("x16", [T, H], FP16, kind="ExternalInput")
    xTc = nc.dram_tensor("xTc", [128, KT * TOUT], FP16, kind="ExternalInput")
    w13 = nc.dram_tensor("w13", [EPC, MT, 128, KT * 256], FP16, kind="ExternalInput")
    w2b = nc.dram_tensor("w2b", [EPC, 4, 128, MT * 512], FP16, kind="ExternalInput")
    sw13 = nc.dram_tensor("sw13", [SIT, 128, KT * 256], FP16, kind="ExternalInput")
    sw2b = nc.dram_tensor("sw2b", [2 * NSTR, 128, SIT * 256], FP16, kind="ExternalInput")
    idxg = nc.dram_tensor("idxg", [EPC, 128, CAPM // 16], I16, kind="ExternalInput")
    idxsa = nc.dram_tensor("idxsa", [EPC, 128, PREM // 16], I16, kind="ExternalInput")
    idxsb = nc.dram_tensor("idxsb", [EPC, 128, SUFM // 16], I16, kind="ExternalInput")
    gat = nc.dram_tensor("gat", [EPC, 128, CTM], F32, kind="ExternalInput")
    out = nc.dram_tensor("out", [TOUT, H], FP16, kind="ExternalOutput")

    y_a = nc.dram_tensor("y_a", [RA + 128, H], FP16)
    y_b = nc.dram_tensor("y_b", [RB + 128, H], FP16)
    rs_a = nc.dram_tensor("rs_a", [RA // NCORES, H], FP16)
    rs_b = nc.dram_tensor("rs_b", [RB // NCORES, H], FP16)
    warm_in = nc.dram_tensor("warm_in", [2048, 512], FP16)
    warm_out = nc.dram_tensor("warm_out", [256, 512], FP16)
    hb_in = [nc.dram_tensor(f"hb_in{e}", [256, 512], FP16) for e in range(EPC)]
    hb_out = [nc.dram_tensor(f"hb_out{e}", [32, 512], FP16) for e in range(EPC)]

    grp = [list(range(NCORES))]

    with tile.TileContext(nc) as tc:
        with (
            tc.tile_pool(name="const", bufs=1) as const,
            tc.tile_pool(name="persist", bufs=1) as persist,
            tc.tile_pool(name="xgtp", bufs=2) as xgtp,
            tc.tile_pool(name="xgtp1", bufs=2) as xgtp1,
            tc.tile_pool(name="gp", bufs=2) as gp,
            tc.tile_pool(name="w13p", bufs=2) as w13p,
            tc.tile_pool(name="w2p", bufs=2) as w2p,
            tc.tile_pool(name="ybp", bufs=2) as ybp,
            tc.tile_pool(name="s13p", bufs=2) as s13p,
            tc.tile_pool(name="s2p", bufs=2) as s2p,
            tc.tile_pool(name="small", bufs=2) as small,
            tc.tile_pool(name="psum", bufs=2, space="PSUM") as psum,
        ):
            # ---------------- constants ------------------------------------
            iga = const.tile([128, EPC, CAPM // 16], I16)
            nc.sync.dma_start(iga[:], idxg.rearrange("e p c -> p e c"))
            isa = const.tile([128, EPC, PREM // 16], I16)
            nc.sync.dma_start(isa[:], idxsa.rearrange("e p c -> p e c"))
            isb = const.tile([128, EPC, SUFM // 16], I16)
            nc.sync.dma_start(isb[:], idxsb.rearrange("e p c -> p e c"))
            ga_sb = const.tile([128, EPC, CTM], F32)
            nc.sync.dma_start(ga_sb[:], gat.rearrange("e p c -> p e c"))
            zt = const.tile([128, 2048], FP16)
            nc.vector.memset(zt[:], 0.0)

            gs = persist.tile([128, SIT, TOUT], FP16)
            xtc_sb = persist.tile([128, KT, TOUT], FP16)

            # ---------------- routed experts -------------------------------
            for e in range(EPC):
                CH = CHUNKS[e]
                CAP, PRE, SUF0 = CAPS[e], PRES[e], SUF0S[e]
                SUFN = SUFNS[e]
                CT = CAP // 128
                xgt_c = []
                c0 = 0
                for ci, cw in enumerate(CH):
                    pool_ci = xgtp if ci == 0 else xgtp1
                    xgt = pool_ci.tile([128, KT, cw], FP16, tag=f"xgt{ci}",
                                       name=f"xgt{ci}")
                    nc.gpsimd.dma_gather(
                        xgt[:], x16[:], iga[:, e, c0 // 16:(c0 + cw) // 16],
                        cw, cw, H, transpose=True)
                    xgt_c.append(xgt)
                    c0 += cw
                if e == 0:
                    # warmup collective: after the first gather so it doesn't
                    # delay it (transpose DMAs serialize with collectives).
                    # warm_in is uninitialized garbage - result is discarded.
                    nc.gpsimd.collective_compute(
                        "ReduceScatter", OP.add, replica_groups=grp,
                        ins=[warm_in[:]], outs=[warm_out[:]])
                g = gp.tile([128, MT, CAPM], FP16, tag="g")
                for m in range(MT):
                    w13ta = w13p.tile([128, KT // 2, 256], FP16, tag="w13ta")
                    w13tb = w13p.tile([128, KT // 2, 256], FP16, tag="w13tb")
                    w13s = w13[e, m].rearrange("p (k c) -> p k c", c=256)
                    nc.sync.dma_start(w13ta[:], w13s[:, :KT // 2, :])
                    nc.scalar.dma_start(w13tb[:], w13s[:, KT // 2:, :])
                    c0 = 0
                    for ci, cw in enumerate(CH):
                        p1 = psum.tile([128, 512], F32, tag="p1")
                        p3 = psum.tile([128, 512], F32, tag="p3")
                        for k in range(KT):
                            wh = w13ta if k < KT // 2 else w13tb
                            nc.tensor.matmul(p1[:, :cw], wh[:, k % (KT // 2), :128],
                                             xgt_c[ci][:, k, :],
                                             start=(k == 0), stop=(k == KT - 1))
                        for k in range(KT):
                            wh = w13ta if k < KT // 2 else w13tb
                            nc.tensor.matmul(p3[:, :cw], wh[:, k % (KT // 2), 128:],
                                             xgt_c[ci][:, k, :],
                                             start=(k == 0), stop=(k == KT - 1))
                        nc.scalar.activation(g[:, m, c0:c0 + cw], p1[:, :cw], act)
                        nc.vector.tensor_tensor(g[:, m, c0:c0 + cw],
                                                g[:, m, c0:c0 + cw],
                                                p3[:, :cw], OP.mult)
                        c0 += cw
                if e == 0:
                    # zero y_a/y_b from the SBUF memset tile: write-only HBM
                    # traffic, emitted after e0's w13 kicks so the first
                    # weight loads aren't stuck behind 16.8MB of zeros.
                    for r in range(0, RA, 256):
                        nc.sync.dma_start(y_a[r:r + 128, :], zt[:])
                        nc.scalar.dma_start(y_a[r + 128:r + 256, :], zt[:])
                    for r in range(0, RB, 256):
                        nc.sync.dma_start(y_b[r:r + 128, :], zt[:])
                        nc.scalar.dma_start(y_b[r + 128:r + 256, :], zt[:])
                if e == 2:
                    # shared-expert input tokens (needed only at the end;
                    # emitted here to keep it off the startup-critical queues)
                    nc.scalar.dma_start(
                        xtc_sb[:], xTc.rearrange("p (k t) -> p k t", t=TOUT))
                # small per-expert heartbeat collective tied to this
                # expert's g: keeps the CC fabric clocked up through the
                # routed phase (a cold fabric ran the pre-RS heartbeat at
                # ~10GB/s and delayed RS_a by >100us)
                nc.scalar.dma_start(
                    hb_in[e].rearrange("(c p) w -> p c w", p=128),
                    g[:].rearrange("p k c -> p (k c)")[:, :1024]
                    .rearrange("p (a b) -> p a b", b=512))
                nc.gpsimd.collective_compute(
                    "ReduceScatter", OP.add, replica_groups=grp,
                    ins=[hb_in[e][:]], outs=[hb_out[e][:]])
                for q in range(4):
                    w2ta = w2p.tile([128, MT // 2, 512], FP16, tag="w2ta")
                    w2tb = w2p.tile([128, MT // 2, 512], FP16, tag="w2tb")
                    w2s = w2b[e, q].rearrange("p (k c) -> p k c", c=512)
                    nc.sync.dma_start(w2ta[:], w2s[:, :MT // 2, :])
                    nc.scalar.dma_start(w2tb[:], w2s[:, MT // 2:, :])
                    yb = ybp.tile([128, CTM, 512], FP16, tag="yb")
                    for ct in range(CT):
                        p4a = psum.tile([128, 512], F32, tag="p4a")
                        for k2 in range(MT):
                            w2h = w2ta if k2 < MT // 2 else w2tb
                            nc.tensor.matmul(p4a[:], g[:, k2, ct * 128:(ct + 1) * 128],
                                             w2h[:, k2 % (MT // 2), :],
                                             start=(k2 == 0), stop=(k2 == MT - 1))
                        nc.vector.tensor_tensor(
                            yb[:, ct, :], p4a[:],
                            ga_sb[:, e, ct:ct + 1].to_broadcast([128, 512]),
                            OP.mult)
                    # prefix -> y_a (rows < RA; overflow entries to pad row RA)
                    nc.gpsimd.dma_scatter_add(
                        y_a[:, q * 512:(q + 1) * 512], yb[:, :PRE // 128, :],
                        isa[:, e, :PRE // 16], PRE, PRE, 512, elem_step=H)
                    # suffix -> y_b (rows >= RA; others to pad row RB)
                    nc.gpsimd.dma_scatter_add(
                        y_b[:, q * 512:(q + 1) * 512], yb[:, SUF0 // 128:CT, :],
                        isb[:, e, :SUFN // 16], SUFN, SUFN, 512, elem_step=H)

            nc.gpsimd.collective_compute(
                "ReduceScatter", OP.add, replica_groups=grp,
                ins=[y_a[0:RA, :]], outs=[rs_a[:]])
            nc.gpsimd.collective_compute(
                "ReduceScatter", OP.add, replica_groups=grp,
                ins=[y_b[0:RB, :]], outs=[rs_b[:]])

            # ---------------- shared experts (own rows) --------------------
            for sm in range(SIT):
                s13 = s13p.tile([128, KT, 256], FP16, tag="s13")
                nc.scalar.dma_start(
                    s13[:], sw13[sm].rearrange("p (k c) -> p k c", c=256))
                p1 = psum.tile([128, 512], F32, tag="p1")
                p3 = psum.tile([128, 512], F32, tag="p3")
                for k in range(KT):
                    nc.tensor.matmul(p1[:, :TOUT], s13[:, k, :128], xtc_sb[:, k, :],
                                     start=(k == 0), stop=(k == KT - 1))
                for k in range(KT):
                    nc.tensor.matmul(p3[:, :TOUT], s13[:, k, 128:], xtc_sb[:, k, :],
                                     start=(k == 0), stop=(k == KT - 1))
                nc.scalar.activation(gs[:, sm, :], p1[:, :TOUT], act)
                nc.vector.tensor_tensor(gs[:, sm, :], gs[:, sm, :], p3[:, :TOUT],
                                        OP.mult)

            # shared out per 256-col strip + combine with rs halves.
            # Process rs_a-dependent rows (ts < TS/2) across all strips first,
            # then rs_b rows, so no add waits on the later collective.
            half_ts = TS // 2
            for rgn in range(2):
                for s in range(2 * NSTR):
                    s2 = s2p.tile([128, SIT, 256], FP16, tag="s2")
                    nc.scalar.dma_start(
                        s2[:], sw2b[s].rearrange("p (k c) -> p k c", c=256))
                    for tsr in range(half_ts):
                        ts = rgn * half_ts + tsr
                        po = psum.tile([128, 256], F32, tag="p4a")
                        for k2 in range(SIT):
                            nc.tensor.matmul(po[:], gs[:, k2, ts * 128:(ts + 1) * 128],
                                             s2[:, k2, :],
                                             start=(k2 == 0), stop=(k2 == SIT - 1))
                        rst = small.tile([128, 256], FP16, tag="rst")
                        if rgn == 0:
                            src = rs_a[tsr * 128:(tsr + 1) * 128,
                                       s * 256:(s + 1) * 256]
                        else:
                            src = rs_b[tsr * 128:(tsr + 1) * 128,
                                       s * 256:(s + 1) * 256]
                        # on sync: a collective-gated DMA kick must not sit in
                        # front of s13/s2 loads on the in-order scalar queue
                        nc.sync.dma_start(rst[:], src)
                        ott = small.tile([128, 256], FP16, tag="ott")
                        nc.vector.tensor_tensor(ott[:], po[:], rst[:], OP.add)
                        nc.sync.dma_start(
                            out[ts * 128:(ts + 1) * 128, s * 256:(s + 1) * 256],
                            ott[:])

    if compile_:
        nc.compile()
    else:
        nc.insert_library_loads()
    return nc


def host_prep(hidden_states, gate_weight, w1, w2, w3, sw1, sw2, sw3):
    B, S, H = hidden_states.shape
    T = B * S
    E, I = w1.shape[0], w1.shape[1]
    SI = sw1.shape[0]
    EPC = E // NCORES
    KT, MT, SIT = H // 128, I // 128, SI // 128
    NSTR = H // 512

    x = np.ascontiguousarray(hidden_states.reshape(T, H), dtype=np.float32)
    combine = _routing(x, gate_weight.astype(np.float32))
    tok_lists = [np.nonzero(combine[:, e])[0] for e in range(E)]
    counts = np.array([len(t) for t in tok_lists])

    # rank-matched expert assignment: sort by count desc; slot j holds ranks
    # [8j, 8j+8); core c gets order[8j + c]. Slot capacity covers its max.
    order = np.argsort(-counts, kind="stable")
    CAPS = [max(256, int(np.ceil(counts[order[8 * j]] / 128) * 128))
            for j in range(EPC)]

    RA = T // 2
    RB = T - RA
    counts_a = np.array([(t < RA).sum() for t in tok_lists])
    PRES, SUF0S = [], []
    for j in range(EPC):
        slot_experts = order[8 * j:8 * j + 8]
        ca = counts_a[slot_experts]
        PRES.append(min(CAPS[j], int(np.ceil(ca.max() / 128) * 128)))
        SUF0S.append(int(ca.min()) // 128 * 128)
    CAPM = max(CAPS)
    CTM = CAPM // 128
    PREM = max(PRES)
    SUFM = max(CAPS[j] - SUF0S[j] for j in range(EPC))

    x16 = x.astype(np.float16)
    xT = x.T  # [H, T] view

    s1 = sw1.T.reshape(KT, 128, SIT, 128).transpose(2, 1, 0, 3)
    s3 = sw3.T.reshape(KT, 128, SIT, 128).transpose(2, 1, 0, 3)
    sw13 = np.ascontiguousarray(
        np.concatenate([s1, s3], axis=-1).reshape(SIT, 128, -1), dtype=np.float16)
    sw2b = np.ascontiguousarray(
        sw2.T.reshape(SIT, 128, 2 * NSTR, 256).transpose(2, 1, 0, 3)
        .reshape(2 * NSTR, 128, -1), dtype=np.float16)

    in_maps = []
    for c in range(NCORES):
        els = [int(order[8 * j + c]) for j in range(EPC)]
        idxg = np.zeros((EPC, 128, CAPM // 16), np.int16)
        idxsa = np.zeros((EPC, 128, PREM // 16), np.int16)
        idxsb = np.zeros((EPC, 128, SUFM // 16), np.int16)
        gatv = np.zeros((EPC, 128, CTM), np.float32)
        for j, e in enumerate(els):
            CAP, PRE, SUF0 = CAPS[j], PRES[j], SUF0S[j]
            SUFN = CAP - SUF0
            toks = tok_lists[e]
            n = len(toks)
            na = int(counts_a[e])
            full = np.zeros(CAP, np.int16)
            full[:n] = toks
            idxg[j, :, :CAP // 16] = _wrap16(full)
            # prefix scatter: rows < RA keep their index, others -> pad row RA
            sa = np.full(PRE, RA, np.int16)
            sa[:na] = toks[:na]
            idxsa[j, :, :PRE // 16] = _wrap16(sa)
            # suffix scatter: rows >= RA -> local index, others -> pad row RB
            sb = np.full(SUFN, RB, np.int16)
            sel = full[SUF0:].astype(np.int64)
            vmask = (np.arange(SUF0, CAP) < n) & (sel >= RA)
            sb[vmask] = (sel[vmask] - RA).astype(np.int16)
            idxsb[j, :, :SUFN // 16] = _wrap16(sb)
            gv = np.zeros(CAP, np.float32)
            gv[:n] = combine[toks, e]
            gatv[j, :, :CAP // 128] = gv.reshape(-1, 128).T
        w13c = np.empty((EPC, MT, 128, KT * 256), np.float16)
        w2c = np.empty((EPC, 4, 128, MT * 512), np.float16)
        for j, e in enumerate(els):
            a1 = w1[e].T.reshape(KT, 128, MT, 128).transpose(2, 1, 0, 3)
            a3 = w3[e].T.reshape(KT, 128, MT, 128).transpose(2, 1, 0, 3)
            w13c[j] = np.concatenate([a1, a3], axis=-1).reshape(MT, 128, -1)
            w2c[j] = (w2[e].T.reshape(MT, 128, 4, 512)
                      .transpose(2, 1, 0, 3).reshape(4, 128, -1))
        own_rows = np.concatenate([
            np.arange(c * RA // NCORES, (c + 1) * RA // NCORES),
            np.arange(RA + c * RB // NCORES, RA + (c + 1) * RB // NCORES)])
        xTc = np.ascontiguousarray(
            xT[:, own_rows].reshape(KT, 128, len(own_rows))
            .transpose(1, 0, 2).reshape(128, -1), dtype=np.float16)
        in_maps.append({
            "x16": x16, "xTc": xTc,
            "w13": w13c, "w2b": w2c,
            "sw13": sw13, "sw2b": sw2b,
            "idxg": idxg, "idxsa": idxsa, "idxsb": idxsb,
            "gat": gatv,
        })
    cfg = dict(T=T, H=H, I=I, EPC=EPC, CAPS=CAPS, PRES=PRES, SUF0S=SUF0S,
               RA=RA, SI=SI)
    return in_maps, cfg


def kernel(**inputs):
    inputs = {k: np.asarray(v) for k, v in inputs.items()}
    hs = inputs["hidden_states"]
    B, S, H = hs.shape
    in_maps, cfg = host_prep(
        hs, inputs["gate_weight"], inputs["w1"], inputs["w2"], inputs["w3"],
        inputs["sw1"], inputs["sw2"], inputs["sw3"])
    nc = build_kernel(**cfg)
    res = run_bass_kernel_spmd(nc, in_maps, list(range(NCORES)))
    T = B * S
    RA = cfg["RA"]
    RB = T - RA
    y = np.empty((T, H), np.float32)
    for c in range(NCORES):
        o = res.results[c]["out"]
        y[c * RA // NCORES:(c + 1) * RA // NCORES] = o[:RA // NCORES]
        y[RA + c * RB // NCORES:RA + (c + 1) * RB // NCORES] = o[RA // NCORES:]
    return y.reshape(B, S, H).astype(np.float32)


if __name__ == "__main__":
    pass
